# revision 1
# baseline (speedup 1.0000x reference)
"""CrossAttention Trainium2 kernel, SPMD over 8 NeuronCores.

Problem: x[4,2048,1024], context[4,1024,768], Wq[1024,512], Wk/Wv[768,512],
Wout[512,1024], bout[1024] -> out[4,2048,1024] (f32).

Sharding: 8 cores = 4 batches x 2 halves of the query dim n (2048 -> 2x1024).
Each core computes full attention for its (batch, n-half) with no collectives
(K/V projections are recomputed per half; ~16% extra flops).

Layout strategy: the host feeds x and context pre-transposed (feature-major)
in bf16, so every matmul contraction dim lands on SBUF partitions without any
on-chip transposes:
  qT[inner,n]   = Wq^T @ xT          (lhsT=Wq, rhs=xT)
  kT[inner,m]   = Wk^T @ ctxT        (lhsT=Wk, rhs=ctxT)
  v[m,inner]    = ctx @ Wv           (lhsT=ctxT, rhs=Wv), stored per-head with
                                     a ones-column appended -> v_ext[m, h, 65]
  simT_h[m,n]   = k_h q_h^T          (lhsT=kT_h, rhs=qT_h; K=64, two heads of a
                                     pair run row-packed in the PE array)
  expT_h        = exp(simT_h/8)      (ScalarE, scale folded into activation)
  oT_ext[65,n]  = v_ext^T @ expT_h   (row 64 = softmax denominators)
  oT_h          = oT_ext[0:64] * recip(sums) (DVE fast reciprocal + K=1
                                     broadcast matmul to spread the row)
  outT[qd,n]    = Wout^T @ oT + bout
The host transposes outT back. f32 accumulation everywhere (PSUM); bf16
operands for 2x TensorE throughput and half the DMA bytes.
"""

import numpy as np
import ml_dtypes

import concourse.bass as bass
import concourse.mybir as mybir
import concourse.tile as tile
from concourse import bacc
from concourse.bass_utils import run_bass_kernel_spmd

BF16 = mybir.dt.bfloat16
F32 = mybir.dt.float32

B, N, QD = 4, 2048, 1024
M, CD = 1024, 768
H, D = 8, 64
INNER = H * D  # 512
NSH = N // 2  # 1024 query rows per core
P = 128
FB = 512  # free-dim block (psum bank = 512 f32)

KQ = QD // P  # 8 contraction tiles for q-proj
KC = CD // P  # 6 contraction tiles for k/v-proj
MI = INNER // P  # 4 inner p-tiles (= head pairs)
NB = NSH // FB  # 2 n blocks
MC = M // P  # 8 m chunks
KO = INNER // P  # 4 contraction tiles for out-proj
QT = QD // P  # 8 out-proj row tiles


def build_nc():
    nc = bacc.Bacc(None)

    xT_d = nc.declare_dram_parameter("xT", [QD, NSH], BF16, isOutput=False)
    ctxT_d = nc.declare_dram_parameter("ctxT", [CD, M], BF16, isOutput=False)
    Wq_d = nc.declare_dram_parameter("Wq", [QD, INNER], BF16, isOutput=False)
    Wk_d = nc.declare_dram_parameter("Wk", [CD, INNER], BF16, isOutput=False)
    Wv_d = nc.declare_dram_parameter("Wv", [CD, INNER], BF16, isOutput=False)
    Wout_d = nc.declare_dram_parameter("Wout", [INNER, QD], BF16, isOutput=False)
    bout_d = nc.declare_dram_parameter("bout", [QT, P, 1], F32, isOutput=False)
    outT_d = nc.declare_dram_parameter("outT", [QD, NSH], F32, isOutput=True)

    from contextlib import ExitStack

    with tile.TileContext(nc) as tc, ExitStack() as ctx:
        persist = ctx.enter_context(tc.tile_pool(name="persist", bufs=1))
        pp_proj = ctx.enter_context(tc.tile_pool(name="pp_proj", bufs=2, space="PSUM"))
        pp_sim = ctx.enter_context(tc.tile_pool(name="pp_sim", bufs=2, space="PSUM"))
        pp_o = ctx.enter_context(tc.tile_pool(name="pp_o", bufs=2, space="PSUM"))
        pp_b = ctx.enter_context(tc.tile_pool(name="pp_b", bufs=1, space="PSUM"))
        pp_out = ctx.enter_context(tc.tile_pool(name="pp_out", bufs=1, space="PSUM"))
        sb_tmp = ctx.enter_context(tc.tile_pool(name="sb_tmp", bufs=3))
        expT_pool = ctx.enter_context(tc.tile_pool(name="expT", bufs=32))

        # ---- load everything ----
        xT_sb = []
        for k in range(KQ):
            t = persist.tile([P, NSH], BF16, tag=f"xT{k}", name=f"xT{k}")
            nc.sync.dma_start(out=t[:], in_=xT_d[k * P : (k + 1) * P, :])
            xT_sb.append(t)
        ctxT_sb = []
        for k in range(KC):
            t = persist.tile([P, M], BF16, tag=f"ctxT{k}", name=f"ctxT{k}")
            nc.sync.dma_start(out=t[:], in_=ctxT_d[k * P : (k + 1) * P, :])
            ctxT_sb.append(t)
        Wq_sb = []
        for k in range(KQ):
            t = persist.tile([P, INNER], BF16, tag=f"Wq{k}", name=f"Wq{k}")
            nc.sync.dma_start(out=t[:], in_=Wq_d[k * P : (k + 1) * P, :])
            Wq_sb.append(t)
        Wk_sb = []
        Wv_sb = []
        for k in range(KC):
            t = persist.tile([P, INNER], BF16, tag=f"Wk{k}", name=f"Wk{k}")
            nc.sync.dma_start(out=t[:], in_=Wk_d[k * P : (k + 1) * P, :])
            Wk_sb.append(t)
            t2 = persist.tile([P, INNER], BF16, tag=f"Wv{k}", name=f"Wv{k}")
            nc.sync.dma_start(out=t2[:], in_=Wv_d[k * P : (k + 1) * P, :])
            Wv_sb.append(t2)
        Wout_sb = []
        for k in range(KO):
            t = persist.tile([P, QD], BF16, tag=f"Wout{k}", name=f"Wout{k}")
            nc.sync.dma_start(out=t[:], in_=Wout_d[k * P : (k + 1) * P, :])
            Wout_sb.append(t)
        bout_sb = []
        for k in range(QT):
            t = persist.tile([P, 1], F32, tag=f"bout{k}", name=f"bout{k}")
            nc.sync.dma_start(out=t[:], in_=bout_d[k])
            bout_sb.append(t)

        ones64 = persist.tile([1, 64], F32, tag="ones64", name="ones64")
        nc.vector.memset(ones64[:], 1.0)

        vext_sb = []
        for i in range(MC):
            t = persist.tile([P, H, D + 1], BF16, tag=f"vext{i}", name=f"vext{i}")
            nc.vector.memset(t[:, :, D : D + 1], 1.0)
            vext_sb.append(t)

        qT_sb = [
            persist.tile([P, NSH], BF16, tag=f"qT{i}", name=f"qT{i}")
            for i in range(MI)
        ]
        kT_sb = [
            persist.tile([P, M], BF16, tag=f"kT{i}", name=f"kT{i}") for i in range(MI)
        ]
        oT_sb = [
            persist.tile([P, NSH], BF16, tag=f"oT{i}", name=f"oT{i}")
            for i in range(MI)
        ]

        # ---- projections ----
        for mi in range(MI):
            for nb in range(NB):
                ps_q = pp_proj.tile([P, FB], F32, tag="proj", name="ps_q")
                for k in range(KQ):
                    nc.tensor.matmul(
                        ps_q[:],
                        Wq_sb[k][:, mi * P : (mi + 1) * P],
                        xT_sb[k][:, nb * FB : (nb + 1) * FB],
                        start=(k == 0),
                        stop=(k == KQ - 1),
                    )
                nc.any.tensor_copy(qT_sb[mi][:, nb * FB : (nb + 1) * FB], ps_q[:])
        for mi in range(MI):
            for nb in range(M // FB):
                ps_k = pp_proj.tile([P, FB], F32, tag="proj", name="ps_k")
                for k in range(KC):
                    nc.tensor.matmul(
                        ps_k[:],
                        Wk_sb[k][:, mi * P : (mi + 1) * P],
                        ctxT_sb[k][:, nb * FB : (nb + 1) * FB],
                        start=(k == 0),
                        stop=(k == KC - 1),
                    )
                nc.any.tensor_copy(kT_sb[mi][:, nb * FB : (nb + 1) * FB], ps_k[:])
        for t_i in range(MC):
            ps_v = pp_proj.tile([P, FB], F32, tag="proj", name="ps_v")
            for k in range(KC):
                nc.tensor.matmul(
                    ps_v[:],
                    ctxT_sb[k][:, t_i * P : (t_i + 1) * P],
                    Wv_sb[k][:],
                    start=(k == 0),
                    stop=(k == KC - 1),
                )
            nc.any.tensor_copy(
                vext_sb[t_i][:, :, 0:D],
                ps_v[:].rearrange("p (h d) -> p h d", h=H),
            )

        # ---- attention, one head-pair at a time ----
        for pair in range(MI):
            # simT + exp for both heads of the pair
            exp_t = [
                [
                    expT_pool.tile([P, NSH], BF16, tag="expT", name=f"exp{pair}_{j}_{mc}")
                    for mc in range(MC)
                ]
                for j in range(2)
            ]
            for mc in range(MC):
                for nb in range(NB):
                    for j in range(2):
                        ps_s = pp_sim.tile([P, FB], F32, tag="sim", name="ps_s")
                        nc.tensor.matmul(
                            ps_s[:],
                            kT_sb[pair][j * D : (j + 1) * D, mc * P : (mc + 1) * P],
                            qT_sb[pair][j * D : (j + 1) * D, nb * FB : (nb + 1) * FB],
                            start=True,
                            stop=True,
                        )
                        nc.scalar.activation(
                            exp_t[j][mc][:, nb * FB : (nb + 1) * FB],
                            ps_s[:],
                            mybir.ActivationFunctionType.Exp,
                            scale=float(D) ** -0.5,
                        )
            # oT for both heads
            for j in range(2):
                h = 2 * pair + j
                for nb in range(NB):
                    ps_o = pp_o.tile([D + 1, FB], F32, tag="oT", name="ps_o")
                    for mc in range(MC):
                        nc.tensor.matmul(
                            ps_o[:],
                            vext_sb[mc][:, h : h + 1, :],
                            exp_t[j][mc][:, nb * FB : (nb + 1) * FB],
                            start=(mc == 0),
                            stop=(mc == MC - 1),
                        )
                    sums = sb_tmp.tile([1, FB], F32, tag="sums", name="sums")
                    nc.vector.tensor_copy(sums[:], ps_o[D : D + 1, :])
                    recip = sb_tmp.tile([1, FB], F32, tag="recip", name="recip")
                    nc.vector.reciprocal_approx_fast(out=recip[:], in_=sums[:])
                    ps_rb = pp_b.tile([D, FB], F32, tag="rb", name="ps_rb")
                    nc.tensor.matmul(
                        ps_rb[:], ones64[:], recip[:], start=True, stop=True
                    )
                    recipB = sb_tmp.tile([D, FB], F32, tag="recipB", name="recipB")
                    nc.vector.tensor_copy(recipB[:], ps_rb[:])
                    nc.vector.tensor_mul(
                        oT_sb[pair][j * D : (j + 1) * D, nb * FB : (nb + 1) * FB],
                        ps_o[0:D, :],
                        recipB[:],
                    )

        # ---- output projection + bias ----
        for mi in range(QT):
            for nb in range(NB):
                ps_out = pp_out.tile([P, FB], F32, tag="out", name="ps_out")
                for k in range(KO):
                    nc.tensor.matmul(
                        ps_out[:],
                        Wout_sb[k][:, mi * P : (mi + 1) * P],
                        oT_sb[k][:, nb * FB : (nb + 1) * FB],
                        start=(k == 0),
                        stop=(k == KO - 1),
                    )
                stage = sb_tmp.tile([P, FB], F32, tag="outstage", name="stage")
                nc.scalar.add(stage[:], ps_out[:], bout_sb[mi][:])
                nc.sync.dma_start(
                    out=outT_d[mi * P : (mi + 1) * P, nb * FB : (nb + 1) * FB],
                    in_=stage[:],
                )

    nc.compile()
    return nc


_NC_CACHE = None


def _get_nc():
    global _NC_CACHE
    if _NC_CACHE is None:
        _NC_CACHE = build_nc()
    return _NC_CACHE


def make_in_maps(x, context, Wq, Wk, Wv, Wout, bout):
    bf = ml_dtypes.bfloat16
    Wq_b = np.ascontiguousarray(Wq).astype(bf)
    Wk_b = np.ascontiguousarray(Wk).astype(bf)
    Wv_b = np.ascontiguousarray(Wv).astype(bf)
    Wout_b = np.ascontiguousarray(Wout).astype(bf)
    bout_r = np.ascontiguousarray(bout, dtype=np.float32).reshape(QT, P, 1)
    in_maps = []
    for c in range(8):
        b, half = divmod(c, 2)
        xT = x[b].T[:, half * NSH : (half + 1) * NSH].astype(bf)
        ctxT = context[b].T.astype(bf)
        in_maps.append(
            {
                "xT": xT,
                "ctxT": ctxT,
                "Wq": Wq_b,
                "Wk": Wk_b,
                "Wv": Wv_b,
                "Wout": Wout_b,
                "bout": bout_r,
            }
        )
    return in_maps


def gather_out(results):
    out = np.empty((B, N, QD), dtype=np.float32)
    for c in range(8):
        b, half = divmod(c, 2)
        out[b, half * NSH : (half + 1) * NSH, :] = results[c]["outT"].T
    return out


def kernel(**inputs):
    nc = _get_nc()
    in_maps = make_in_maps(**inputs)
    res = run_bass_kernel_spmd(nc, in_maps, list(range(8)))
    return gather_out(res.results)


if __name__ == "__main__":
    rng = np.random.default_rng(0)
    ins = {
        "x": rng.standard_normal((B, N, QD), dtype=np.float32),
        "context": rng.standard_normal((B, M, CD), dtype=np.float32),
        "Wq": rng.standard_normal((QD, INNER), dtype=np.float32) / 32,
        "Wk": rng.standard_normal((CD, INNER), dtype=np.float32) / 27.7,
        "Wv": rng.standard_normal((CD, INNER), dtype=np.float32) / 27.7,
        "Wout": rng.standard_normal((INNER, QD), dtype=np.float32) / 22.6,
        "bout": rng.standard_normal((QD,), dtype=np.float32) * 0.01,
    }
    out = kernel(**ins)
    print("out", out.shape, out.dtype, np.abs(out).mean())


# revision 5
# speedup vs baseline: 1.2062x; 1.2062x over previous
"""CrossAttention Trainium2 kernel, SPMD over 8 NeuronCores.

Problem: x[4,2048,1024], context[4,1024,768], Wq[1024,512], Wk/Wv[768,512],
Wout[512,1024], bout[1024] -> out[4,2048,1024] (f32).

Sharding: 8 cores = 4 batches x 2 halves of the query dim n (2048 -> 2x1024).
Each core computes full attention for its (batch, n-half) with no collectives
(K/V projections are recomputed per half; ~16% extra flops).

Layout strategy: the host feeds x and context pre-transposed (feature-major)
in bf16, so every matmul contraction dim lands on SBUF partitions without any
on-chip transposes:
  qT[inner,n]   = Wq^T @ xT          (lhsT=Wq, rhs=xT)
  kT[inner,m]   = Wk^T @ ctxT        (lhsT=Wk, rhs=ctxT)
  v[m,inner]    = ctx @ Wv           (lhsT=ctxT, rhs=Wv), stored per-head with
                                     a ones-column appended -> v_ext[m, h, 65]
  simT_h[m,n]   = k_h q_h^T          (lhsT=kT_h, rhs=qT_h; K=64, two heads of a
                                     pair run row-packed in the PE array)
  expT_h        = exp(simT_h/8)      (ScalarE, scale folded into activation)
  oT_ext[65,n]  = v_ext^T @ expT_h   (row 64 = softmax denominators)
  oT_h          = oT_ext[0:64] * recip(sums) (DVE fast reciprocal + K=1
                                     broadcast matmul to spread the row)
  outT[qd,n]    = Wout^T @ oT + bout
The host transposes outT back. f32 accumulation everywhere (PSUM); bf16
operands for 2x TensorE throughput and half the DMA bytes.
"""

import numpy as np
import ml_dtypes

import concourse.bass as bass
import concourse.mybir as mybir
import concourse.tile as tile
from concourse import bacc
from concourse.bass_utils import run_bass_kernel_spmd

BF16 = mybir.dt.bfloat16
F32 = mybir.dt.float32

B, N, QD = 4, 2048, 1024
M, CD = 1024, 768
H, D = 8, 64
INNER = H * D  # 512
NSH = N // 2  # 1024 query rows per core
P = 128
FB = 512  # free-dim block (psum bank = 512 f32)

KQ = QD // P  # 8 contraction tiles for q-proj
KC = CD // P  # 6 contraction tiles for k/v-proj
MI = INNER // P  # 4 inner p-tiles (= head pairs)
NB = NSH // FB  # 2 n blocks
MC = M // P  # 8 m chunks
KO = INNER // P  # 4 contraction tiles for out-proj
QT = QD // P  # 8 out-proj row tiles


def build_nc():
    nc = bacc.Bacc(None)

    xT_d = nc.declare_dram_parameter("xT", [QD, NSH], BF16, isOutput=False)
    ctxT_d = nc.declare_dram_parameter("ctxT", [CD, M], BF16, isOutput=False)
    Wq_d = nc.declare_dram_parameter("Wq", [QD, INNER], BF16, isOutput=False)
    Wk_d = nc.declare_dram_parameter("Wk", [CD, INNER], BF16, isOutput=False)
    Wv_d = nc.declare_dram_parameter("Wv", [CD, INNER], BF16, isOutput=False)
    Wout_d = nc.declare_dram_parameter("Wout", [INNER, QD], BF16, isOutput=False)
    bout_d = nc.declare_dram_parameter("bout", [QT, P, 1], F32, isOutput=False)
    outT_d = nc.declare_dram_parameter("outT", [QD, NSH], F32, isOutput=True)

    from contextlib import ExitStack

    with tile.TileContext(nc) as tc, ExitStack() as ctx:
        persist = ctx.enter_context(tc.tile_pool(name="persist", bufs=1))
        # PSUM budget (8 banks): sim 2x[128,1024]=4, o/rb shared 2, proj/out 2
        pp_mm = ctx.enter_context(tc.tile_pool(name="pp_mm", bufs=2, space="PSUM"))
        pp_sim = ctx.enter_context(tc.tile_pool(name="pp_sim", bufs=2, space="PSUM"))
        pp_ob = ctx.enter_context(tc.tile_pool(name="pp_ob", bufs=2, space="PSUM"))
        sb_tmp = ctx.enter_context(tc.tile_pool(name="sb_tmp", bufs=3))
        expT_pool = ctx.enter_context(tc.tile_pool(name="expT", bufs=32))

        # ---- load everything (q-proj inputs first so PE starts earliest) ----
        xT_sb = []
        Wq_sb = []
        for k in range(KQ):
            t = persist.tile([P, NSH], BF16, tag=f"xT{k}", name=f"xT{k}")
            nc.sync.dma_start(out=t[:], in_=xT_d[k * P : (k + 1) * P, :])
            xT_sb.append(t)
            t2 = persist.tile([P, INNER], BF16, tag=f"Wq{k}", name=f"Wq{k}")
            nc.sync.dma_start(out=t2[:], in_=Wq_d[k * P : (k + 1) * P, :])
            Wq_sb.append(t2)
        ctxT_sb = []
        for k in range(KC):
            t = persist.tile([P, M], BF16, tag=f"ctxT{k}", name=f"ctxT{k}")
            nc.sync.dma_start(out=t[:], in_=ctxT_d[k * P : (k + 1) * P, :])
            ctxT_sb.append(t)
        Wk_sb = []
        Wv_sb = []
        for k in range(KC):
            t = persist.tile([P, INNER], BF16, tag=f"Wk{k}", name=f"Wk{k}")
            nc.sync.dma_start(out=t[:], in_=Wk_d[k * P : (k + 1) * P, :])
            Wk_sb.append(t)
            t2 = persist.tile([P, INNER], BF16, tag=f"Wv{k}", name=f"Wv{k}")
            nc.sync.dma_start(out=t2[:], in_=Wv_d[k * P : (k + 1) * P, :])
            Wv_sb.append(t2)
        Wout_sb = []
        for k in range(KO):
            t = persist.tile([P, QD], BF16, tag=f"Wout{k}", name=f"Wout{k}")
            nc.sync.dma_start(out=t[:], in_=Wout_d[k * P : (k + 1) * P, :])
            Wout_sb.append(t)
        bout_sb = []
        for k in range(QT):
            t = persist.tile([P, 1], F32, tag=f"bout{k}", name=f"bout{k}")
            nc.sync.dma_start(out=t[:], in_=bout_d[k])
            bout_sb.append(t)

        ones64 = persist.tile([1, 64], F32, tag="ones64", name="ones64")
        nc.vector.memset(ones64[:], 1.0)

        vext_sb = []
        for i in range(MC):
            t = persist.tile([P, H, D + 1], BF16, tag=f"vext{i}", name=f"vext{i}")
            nc.vector.memset(t[:, :, D : D + 1], 1.0)
            vext_sb.append(t)

        qT_sb = [
            persist.tile([P, NSH], BF16, tag=f"qT{i}", name=f"qT{i}")
            for i in range(MI)
        ]
        kT_sb = [
            persist.tile([P, M], BF16, tag=f"kT{i}", name=f"kT{i}") for i in range(MI)
        ]
        oT_sb = [
            persist.tile([P, NSH], BF16, tag=f"oT{i}", name=f"oT{i}")
            for i in range(MI)
        ]

        # ---- projections ----
        for mi in range(MI):
            for nb in range(NB):
                ps_q = pp_mm.tile([P, FB], F32, tag="mm", name="ps_q")
                for k in range(KQ):
                    nc.tensor.matmul(
                        ps_q[:],
                        Wq_sb[k][:, mi * P : (mi + 1) * P],
                        xT_sb[k][:, nb * FB : (nb + 1) * FB],
                        start=(k == 0),
                        stop=(k == KQ - 1),
                    )
                nc.vector.tensor_copy(qT_sb[mi][:, nb * FB : (nb + 1) * FB], ps_q[:])
        for mi in range(MI):
            for nb in range(M // FB):
                ps_k = pp_mm.tile([P, FB], F32, tag="mm", name="ps_k")
                for k in range(KC):
                    nc.tensor.matmul(
                        ps_k[:],
                        Wk_sb[k][:, mi * P : (mi + 1) * P],
                        ctxT_sb[k][:, nb * FB : (nb + 1) * FB],
                        start=(k == 0),
                        stop=(k == KC - 1),
                    )
                nc.vector.tensor_copy(kT_sb[mi][:, nb * FB : (nb + 1) * FB], ps_k[:])
        for t_i in range(MC):
            ps_v = pp_mm.tile([P, FB], F32, tag="mm", name="ps_v")
            for k in range(KC):
                nc.tensor.matmul(
                    ps_v[:],
                    ctxT_sb[k][:, t_i * P : (t_i + 1) * P],
                    Wv_sb[k][:],
                    start=(k == 0),
                    stop=(k == KC - 1),
                )
            nc.vector.tensor_copy(
                vext_sb[t_i][:, :, 0:D],
                ps_v[:].rearrange("p (h d) -> p h d", h=H),
            )

        # ---- attention, one head-pair at a time ----
        for pair in range(MI):
            # simT + exp for both heads of the pair; one [128,1024] psum
            # (2 banks) per (mc, j) so exp runs as a single big ACTIVATE
            exp_t = [
                [
                    expT_pool.tile([P, NSH], BF16, tag="expT", name=f"exp{pair}_{j}_{mc}")
                    for mc in range(MC)
                ]
                for j in range(2)
            ]
            for mc in range(MC):
                for j in range(2):
                    ps_s = pp_sim.tile([P, NSH], F32, tag="sim", name="ps_s")
                    for nb in range(NB):
                        nc.tensor.matmul(
                            ps_s[:, nb * FB : (nb + 1) * FB],
                            kT_sb[pair][j * D : (j + 1) * D, mc * P : (mc + 1) * P],
                            qT_sb[pair][j * D : (j + 1) * D, nb * FB : (nb + 1) * FB],
                            start=True,
                            stop=True,
                        )
                    nc.scalar.activation(
                        exp_t[j][mc][:],
                        ps_s[:],
                        mybir.ActivationFunctionType.Exp,
                        scale=float(D) ** -0.5,
                    )
            # oT for both heads
            for j in range(2):
                h = 2 * pair + j
                for nb in range(NB):
                    ps_o = pp_ob.tile([D + 1, FB], F32, tag="ob", name="ps_o")
                    for mc in range(MC):
                        nc.tensor.matmul(
                            ps_o[:],
                            vext_sb[mc][:, h : h + 1, :],
                            exp_t[j][mc][:, nb * FB : (nb + 1) * FB],
                            start=(mc == 0),
                            stop=(mc == MC - 1),
                        )
                    sums = sb_tmp.tile([1, FB], F32, tag="sums", name="sums")
                    nc.vector.tensor_copy(sums[:], ps_o[D : D + 1, :])
                    recip = sb_tmp.tile([1, FB], F32, tag="recip", name="recip")
                    nc.vector.reciprocal_approx_fast(out=recip[:], in_=sums[:])
                    ps_rb = pp_ob.tile([D, FB], F32, tag="ob", name="ps_rb")
                    nc.tensor.matmul(
                        ps_rb[:], ones64[:], recip[:], start=True, stop=True
                    )
                    recipB = sb_tmp.tile([D, FB], F32, tag="recipB", name="recipB")
                    nc.vector.tensor_copy(recipB[:], ps_rb[:])
                    nc.vector.tensor_mul(
                        oT_sb[pair][j * D : (j + 1) * D, nb * FB : (nb + 1) * FB],
                        ps_o[0:D, :],
                        recipB[:],
                    )

        # ---- output projection + bias ----
        for mi in range(QT):
            for nb in range(NB):
                ps_out = pp_mm.tile([P, FB], F32, tag="mm", name="ps_out")
                for k in range(KO):
                    nc.tensor.matmul(
                        ps_out[:],
                        Wout_sb[k][:, mi * P : (mi + 1) * P],
                        oT_sb[k][:, nb * FB : (nb + 1) * FB],
                        start=(k == 0),
                        stop=(k == KO - 1),
                    )
                stage = sb_tmp.tile([P, FB], F32, tag="outstage", name="stage")
                nc.vector.tensor_scalar_add(stage[:], ps_out[:], bout_sb[mi][:])
                nc.sync.dma_start(
                    out=outT_d[mi * P : (mi + 1) * P, nb * FB : (nb + 1) * FB],
                    in_=stage[:],
                )

    nc.compile()
    return nc


_NC_CACHE = None


def _get_nc():
    global _NC_CACHE
    if _NC_CACHE is None:
        _NC_CACHE = build_nc()
    return _NC_CACHE


def make_in_maps(x, context, Wq, Wk, Wv, Wout, bout):
    bf = ml_dtypes.bfloat16
    Wq_b = np.ascontiguousarray(Wq).astype(bf)
    Wk_b = np.ascontiguousarray(Wk).astype(bf)
    Wv_b = np.ascontiguousarray(Wv).astype(bf)
    Wout_b = np.ascontiguousarray(Wout).astype(bf)
    bout_r = np.ascontiguousarray(bout, dtype=np.float32).reshape(QT, P, 1)
    in_maps = []
    for c in range(8):
        b, half = divmod(c, 2)
        xT = x[b].T[:, half * NSH : (half + 1) * NSH].astype(bf)
        ctxT = context[b].T.astype(bf)
        in_maps.append(
            {
                "xT": xT,
                "ctxT": ctxT,
                "Wq": Wq_b,
                "Wk": Wk_b,
                "Wv": Wv_b,
                "Wout": Wout_b,
                "bout": bout_r,
            }
        )
    return in_maps


def gather_out(results):
    out = np.empty((B, N, QD), dtype=np.float32)
    for c in range(8):
        b, half = divmod(c, 2)
        out[b, half * NSH : (half + 1) * NSH, :] = results[c]["outT"].T
    return out


def kernel(**inputs):
    nc = _get_nc()
    in_maps = make_in_maps(**inputs)
    res = run_bass_kernel_spmd(nc, in_maps, list(range(8)))
    return gather_out(res.results)


if __name__ == "__main__":
    rng = np.random.default_rng(0)
    ins = {
        "x": rng.standard_normal((B, N, QD), dtype=np.float32),
        "context": rng.standard_normal((B, M, CD), dtype=np.float32),
        "Wq": rng.standard_normal((QD, INNER), dtype=np.float32) / 32,
        "Wk": rng.standard_normal((CD, INNER), dtype=np.float32) / 27.7,
        "Wv": rng.standard_normal((CD, INNER), dtype=np.float32) / 27.7,
        "Wout": rng.standard_normal((INNER, QD), dtype=np.float32) / 22.6,
        "bout": rng.standard_normal((QD,), dtype=np.float32) * 0.01,
    }
    out = kernel(**ins)
    print("out", out.shape, out.dtype, np.abs(out).mean())


# revision 9
# speedup vs baseline: 1.3931x; 1.1549x over previous
"""CrossAttention Trainium2 kernel, SPMD over 8 NeuronCores.

Problem: x[4,2048,1024], context[4,1024,768], Wq[1024,512], Wk/Wv[768,512],
Wout[512,1024], bout[1024] -> out[4,2048,1024] (f32).

Sharding: 8 cores = 4 batches x 2 halves of the query dim n (2048 -> 2x1024).
Each core computes full attention for its (batch, n-half) with no collectives
(K/V projections are recomputed per half; ~16% extra flops).

Layout strategy: the host feeds x and context pre-transposed (feature-major)
in bf16, so every matmul contraction dim lands on SBUF partitions without any
on-chip transposes:
  qT[inner,n]   = Wq^T @ xT          (lhsT=Wq, rhs=xT)
  kT[inner,m]   = Wk^T @ ctxT        (lhsT=Wk, rhs=ctxT)
  v[m,inner]    = ctx @ Wv           (lhsT=ctxT, rhs=Wv), stored per-head with
                                     a ones-column appended -> v_ext[m, h, 65]
  simT_h[m,n]   = k_h q_h^T          (lhsT=kT_h, rhs=qT_h; K=64, two heads of a
                                     pair run row-packed in the PE array)
  expT_h        = exp(simT_h/8)      (ScalarE, scale folded into activation)
  oT_ext[65,n]  = v_ext^T @ expT_h   (row 64 = softmax denominators)
  oT_h          = oT_ext[0:64] * recip(sums) (DVE fast reciprocal + K=1
                                     broadcast matmul to spread the row)
  outT[qd,n]    = Wout^T @ oT + bout
The host transposes outT back. f32 accumulation everywhere (PSUM); bf16
operands for 2x TensorE throughput and half the DMA bytes.
"""

import numpy as np
import ml_dtypes

import concourse.bass as bass
import concourse.mybir as mybir
import concourse.tile as tile
from concourse import bacc
from concourse.bass_utils import run_bass_kernel_spmd

BF16 = mybir.dt.bfloat16
F32 = mybir.dt.float32

B, N, QD = 4, 2048, 1024
M, CD = 1024, 768
H, D = 8, 64
INNER = H * D  # 512
NSH = N // 2  # 1024 query rows per core
P = 128
FB = 512  # free-dim block (psum bank = 512 f32)

KQ = QD // P  # 8 contraction tiles for q-proj
KC = CD // P  # 6 contraction tiles for k/v-proj
MI = INNER // P  # 4 inner p-tiles (= head pairs)
NB = NSH // FB  # 2 n blocks
MC = M // P  # 8 m chunks
KO = INNER // P  # 4 contraction tiles for out-proj
QT = QD // P  # 8 out-proj row tiles


def build_nc():
    nc = bacc.Bacc(None)

    xT_d = nc.declare_dram_parameter("xT", [QD, NSH], BF16, isOutput=False)
    ctxT_d = nc.declare_dram_parameter("ctxT", [CD, M], BF16, isOutput=False)
    Wq_d = nc.declare_dram_parameter("Wq", [QD, INNER], BF16, isOutput=False)
    Wk_d = nc.declare_dram_parameter("Wk", [CD, INNER], BF16, isOutput=False)
    Wv_d = nc.declare_dram_parameter("Wv", [CD, INNER], BF16, isOutput=False)
    Wout_d = nc.declare_dram_parameter("Wout", [INNER, QD], BF16, isOutput=False)
    bout_d = nc.declare_dram_parameter("bout", [QT, P, 1], F32, isOutput=False)
    outT_d = nc.declare_dram_parameter("outT", [QD, NSH], F32, isOutput=True)

    from contextlib import ExitStack

    with tile.TileContext(nc) as tc, ExitStack() as ctx:
        persist = ctx.enter_context(tc.tile_pool(name="persist", bufs=1))
        # PSUM budget (8 banks): sim 2x[128,1024]=4, o/rb shared 2, proj/out 2
        pp_mm = ctx.enter_context(tc.tile_pool(name="pp_mm", bufs=2, space="PSUM"))
        pp_sim = ctx.enter_context(tc.tile_pool(name="pp_sim", bufs=2, space="PSUM"))
        pp_ob = ctx.enter_context(tc.tile_pool(name="pp_ob", bufs=2, space="PSUM"))
        sb_tmp = ctx.enter_context(tc.tile_pool(name="sb_tmp", bufs=3))
        expT_pool = ctx.enter_context(tc.tile_pool(name="expT", bufs=32))

        # ---- load everything (q-proj inputs first so PE starts earliest) ----
        xT_sb = []
        Wq_sb = []
        for k in range(KQ):
            t = persist.tile([P, NSH], BF16, tag=f"xT{k}", name=f"xT{k}")
            nc.sync.dma_start(out=t[:], in_=xT_d[k * P : (k + 1) * P, :])
            xT_sb.append(t)
            t2 = persist.tile([P, INNER], BF16, tag=f"Wq{k}", name=f"Wq{k}")
            nc.sync.dma_start(out=t2[:], in_=Wq_d[k * P : (k + 1) * P, :])
            Wq_sb.append(t2)
        ctxT_sb = []
        for k in range(KC):
            t = persist.tile([P, M], BF16, tag=f"ctxT{k}", name=f"ctxT{k}")
            nc.sync.dma_start(out=t[:], in_=ctxT_d[k * P : (k + 1) * P, :])
            ctxT_sb.append(t)
        Wk_sb = []
        Wv_sb = []
        for k in range(KC):
            t = persist.tile([P, INNER], BF16, tag=f"Wk{k}", name=f"Wk{k}")
            nc.sync.dma_start(out=t[:], in_=Wk_d[k * P : (k + 1) * P, :])
            Wk_sb.append(t)
            t2 = persist.tile([P, INNER], BF16, tag=f"Wv{k}", name=f"Wv{k}")
            nc.sync.dma_start(out=t2[:], in_=Wv_d[k * P : (k + 1) * P, :])
            Wv_sb.append(t2)
        Wout_sb = []
        for k in range(KO):
            t = persist.tile([P, QD], BF16, tag=f"Wout{k}", name=f"Wout{k}")
            nc.sync.dma_start(out=t[:], in_=Wout_d[k * P : (k + 1) * P, :])
            Wout_sb.append(t)
        bout_sb = []
        for k in range(QT):
            t = persist.tile([P, 1], F32, tag=f"bout{k}", name=f"bout{k}")
            nc.sync.dma_start(out=t[:], in_=bout_d[k])
            bout_sb.append(t)

        ones64 = persist.tile([1, 64], F32, tag="ones64", name="ones64")
        nc.vector.memset(ones64[:], 1.0)

        vext_sb = []
        for i in range(MC):
            t = persist.tile([P, H, D + 1], BF16, tag=f"vext{i}", name=f"vext{i}")
            nc.vector.memset(t[:, :, D : D + 1], 1.0)
            vext_sb.append(t)

        qT_sb = [
            persist.tile([P, NSH], BF16, tag=f"qT{i}", name=f"qT{i}")
            for i in range(MI)
        ]
        kT_sb = [
            persist.tile([P, M], BF16, tag=f"kT{i}", name=f"kT{i}") for i in range(MI)
        ]
        oT_sb = [
            persist.tile([P, NSH], BF16, tag=f"oT{i}", name=f"oT{i}")
            for i in range(MI)
        ]

        # ---- projection emitters (interleaved into the attention pair loop
        # so ready PE work fills the bubbles while ScalarE runs exp) ----
        def proj_q(mi):
            for nb in range(NB):
                ps_q = pp_mm.tile([P, FB], F32, tag="mm", name="ps_q")
                for k in range(KQ):
                    nc.tensor.matmul(
                        ps_q[:],
                        Wq_sb[k][:, mi * P : (mi + 1) * P],
                        xT_sb[k][:, nb * FB : (nb + 1) * FB],
                        start=(k == 0),
                        stop=(k == KQ - 1),
                    )
                nc.vector.tensor_copy(qT_sb[mi][:, nb * FB : (nb + 1) * FB], ps_q[:])

        def proj_k(mi):
            for nb in range(M // FB):
                ps_k = pp_mm.tile([P, FB], F32, tag="mm", name="ps_k")
                for k in range(KC):
                    nc.tensor.matmul(
                        ps_k[:],
                        Wk_sb[k][:, mi * P : (mi + 1) * P],
                        ctxT_sb[k][:, nb * FB : (nb + 1) * FB],
                        start=(k == 0),
                        stop=(k == KC - 1),
                    )
                nc.vector.tensor_copy(kT_sb[mi][:, nb * FB : (nb + 1) * FB], ps_k[:])

        def proj_v(t_i):
            ps_v = pp_mm.tile([P, FB], F32, tag="mm", name="ps_v")
            for k in range(KC):
                nc.tensor.matmul(
                    ps_v[:],
                    ctxT_sb[k][:, t_i * P : (t_i + 1) * P],
                    Wv_sb[k][:],
                    start=(k == 0),
                    stop=(k == KC - 1),
                )
            nc.vector.tensor_copy(
                vext_sb[t_i][:, :, 0:D],
                ps_v[:].rearrange("p (h d) -> p h d", h=H),
            )

        # prework: only what pair 0 needs, plus all of v
        proj_q(0)
        proj_k(0)
        for t_i in range(MC):
            proj_v(t_i)

        # ---- attention, one head-pair at a time ----
        for pair in range(MI):
            # simT + exp for both heads of the pair; one [128,1024] psum
            # (2 banks) per (mc, j) so exp runs as a single big ACTIVATE
            exp_t = [
                [
                    expT_pool.tile([P, NSH], BF16, tag="expT", name=f"exp{pair}_{j}_{mc}")
                    for mc in range(MC)
                ]
                for j in range(2)
            ]
            for mc in range(MC):
                for j in range(2):
                    ps_s = pp_sim.tile([P, NSH], F32, tag="sim", name="ps_s")
                    for nb in range(NB):
                        nc.tensor.matmul(
                            ps_s[:, nb * FB : (nb + 1) * FB],
                            kT_sb[pair][j * D : (j + 1) * D, mc * P : (mc + 1) * P],
                            qT_sb[pair][j * D : (j + 1) * D, nb * FB : (nb + 1) * FB],
                            start=True,
                            stop=True,
                        )
                    nc.scalar.activation(
                        exp_t[j][mc][:],
                        ps_s[:],
                        mybir.ActivationFunctionType.Exp,
                        scale=float(D) ** -0.5,
                    )
            # next pair's projections: ready PE work to fill exp-stall bubbles
            if pair + 1 < MI:
                proj_q(pair + 1)
                proj_k(pair + 1)
            # oT for both heads
            for j in range(2):
                h = 2 * pair + j
                for nb in range(NB):
                    ps_o = pp_ob.tile([D + 1, FB], F32, tag="ob", name="ps_o")
                    for mc in range(MC):
                        nc.tensor.matmul(
                            ps_o[:],
                            vext_sb[mc][:, h : h + 1, :],
                            exp_t[j][mc][:, nb * FB : (nb + 1) * FB],
                            start=(mc == 0),
                            stop=(mc == MC - 1),
                        )
                    sums = sb_tmp.tile([1, FB], F32, tag="sums", name="sums")
                    nc.vector.tensor_copy(sums[:], ps_o[D : D + 1, :])
                    recip = sb_tmp.tile([1, FB], F32, tag="recip", name="recip")
                    nc.vector.reciprocal_approx_fast(out=recip[:], in_=sums[:])
                    ps_rb = pp_mm.tile([D, FB], F32, tag="mm", name="ps_rb")
                    nc.tensor.matmul(
                        ps_rb[:], ones64[:], recip[:], start=True, stop=True
                    )
                    recipB = sb_tmp.tile([D, FB], F32, tag="recipB", name="recipB")
                    nc.vector.tensor_copy(recipB[:], ps_rb[:])
                    nc.vector.tensor_mul(
                        oT_sb[pair][j * D : (j + 1) * D, nb * FB : (nb + 1) * FB],
                        ps_o[0:D, :],
                        recipB[:],
                    )

        # ---- output projection + bias ----
        for nb in range(NB):
            for mi in range(QT):
                ps_out = pp_mm.tile([P, FB], F32, tag="mm", name="ps_out")
                for k in range(KO):
                    nc.tensor.matmul(
                        ps_out[:],
                        Wout_sb[k][:, mi * P : (mi + 1) * P],
                        oT_sb[k][:, nb * FB : (nb + 1) * FB],
                        start=(k == 0),
                        stop=(k == KO - 1),
                    )
                stage = sb_tmp.tile([P, FB], F32, tag="outstage", name="stage")
                nc.vector.tensor_scalar_add(stage[:], ps_out[:], bout_sb[mi][:])
                nc.sync.dma_start(
                    out=outT_d[mi * P : (mi + 1) * P, nb * FB : (nb + 1) * FB],
                    in_=stage[:],
                )

    nc.compile()
    return nc


_NC_CACHE = None


def _get_nc():
    global _NC_CACHE
    if _NC_CACHE is None:
        _NC_CACHE = build_nc()
    return _NC_CACHE


def make_in_maps(x, context, Wq, Wk, Wv, Wout, bout):
    bf = ml_dtypes.bfloat16
    Wq_b = np.ascontiguousarray(Wq).astype(bf)
    Wk_b = np.ascontiguousarray(Wk).astype(bf)
    Wv_b = np.ascontiguousarray(Wv).astype(bf)
    Wout_b = np.ascontiguousarray(Wout).astype(bf)
    bout_r = np.ascontiguousarray(bout, dtype=np.float32).reshape(QT, P, 1)
    in_maps = []
    for c in range(8):
        b, half = divmod(c, 2)
        xT = x[b].T[:, half * NSH : (half + 1) * NSH].astype(bf)
        ctxT = context[b].T.astype(bf)
        in_maps.append(
            {
                "xT": xT,
                "ctxT": ctxT,
                "Wq": Wq_b,
                "Wk": Wk_b,
                "Wv": Wv_b,
                "Wout": Wout_b,
                "bout": bout_r,
            }
        )
    return in_maps


def gather_out(results):
    out = np.empty((B, N, QD), dtype=np.float32)
    for c in range(8):
        b, half = divmod(c, 2)
        out[b, half * NSH : (half + 1) * NSH, :] = results[c]["outT"].T
    return out


def kernel(**inputs):
    nc = _get_nc()
    in_maps = make_in_maps(**inputs)
    res = run_bass_kernel_spmd(nc, in_maps, list(range(8)))
    return gather_out(res.results)


if __name__ == "__main__":
    rng = np.random.default_rng(0)
    ins = {
        "x": rng.standard_normal((B, N, QD), dtype=np.float32),
        "context": rng.standard_normal((B, M, CD), dtype=np.float32),
        "Wq": rng.standard_normal((QD, INNER), dtype=np.float32) / 32,
        "Wk": rng.standard_normal((CD, INNER), dtype=np.float32) / 27.7,
        "Wv": rng.standard_normal((CD, INNER), dtype=np.float32) / 27.7,
        "Wout": rng.standard_normal((INNER, QD), dtype=np.float32) / 22.6,
        "bout": rng.standard_normal((QD,), dtype=np.float32) * 0.01,
    }
    out = kernel(**ins)
    print("out", out.shape, out.dtype, np.abs(out).mean())
